# revision 18
# baseline (speedup 1.0000x reference)
"""DisparityWarping Trainium2 kernel (8-core data parallel).

Full inputs: x [4,16,320,1024] f32, disparity [4,1,320,1024] f32 in [0,64).
Returns (warped [4,16,320,1024] f32, mask [4,1,320,1024] bool).

Sharding: core k -> batch k//2, row-half k%2 (160 rows each).

Per-core algorithm:
  warp[c,i,j] = sum_m x[c,i,m] * hat(gx_pix[i,j] - m),  hat(t)=relu(1-|t|)
    - vertical bilinear is identity (gy_pix == row) to ~1.5e-5.
    - banded matmul: m-blocks of 128; j-window 192 per block.
    - band built on PE: selector matmul broadcasts gxloc = (P_int-128*blk)+frac
      over 128 partitions; ACT Abs with per-partition bias -k gives |t|;
      DVE min/sub gives -hat in fp16; gather matmuls accumulate -out in PSUM.
  mask[i,j] = (|gx|<=1) & not exists k in [2,63]: trans[i,j+k]==trans[i,j],
      trans = max(j - floor(disp), 0)   (62-tap shifted compare on DVE)
"""

import numpy as np

N, C, H, W = 4, 16, 320, 1024
RPC = 160
NB = 8
WIN = 192
BAND = NB * WIN          # 1536
HALF = BAND // 2         # 768
PADW = W + 64            # 1088
NCORES = 8
RGRP = 4
A_BIG = np.float32(8388607.5)
PAD_VAL = -10000.0

_cached = {}


def build_program(nrows=RPC):
    import concourse.bacc as bacc
    import concourse.mybir as mybir
    import concourse.tile as tile
    from concourse.bass import AP

    f32, f16, u8 = mybir.dt.float32, mybir.dt.float16, mybir.dt.uint8
    Alu = mybir.AluOpType
    Act = mybir.ActivationFunctionType

    nc = bacc.Bacc("TRN2", target_bir_lowering=False, debug=False,
                   enable_asserts=False, num_devices=NCORES)

    nslot = (nrows + 31) // 32
    nq = min(32, nrows)
    ngrp = nrows // RGRP
    xfree = nrows * NB * C
    RMP = nslot * BAND   # RM per-partition pitch

    xt_d = nc.dram_tensor("xt", [128, xfree], f32, kind="ExternalInput")
    disp_d = nc.dram_tensor("disp", [nrows, W], f32, kind="ExternalInput")
    iota_d = nc.dram_tensor("iotaj", [128, W], f32, kind="ExternalInput")
    blkc_d = nc.dram_tensor("blkc", [128, BAND], f16, kind="ExternalInput")
    sel_d = nc.dram_tensor("sel", [nq, 64, 128], f16, kind="ExternalInput")
    negk_d = nc.dram_tensor("negk", [128, 1], f32, kind="ExternalInput")
    warp_d = nc.dram_tensor("warped", [C, nrows, W], f32, kind="ExternalOutput")
    mask_d = nc.dram_tensor("mask", [nrows, W], u8, kind="ExternalOutput")

    with tile.TileContext(nc) as tc:
        with tc.tile_pool(name="const", bufs=1) as constp, \
             tc.tile_pool(name="xpool", bufs=1) as xpool, \
             tc.tile_pool(name="xload", bufs=2) as xload, \
             tc.tile_pool(name="prep", bufs=1) as prep, \
             tc.tile_pool(name="band", bufs=3) as bandp, \
             tc.tile_pool(name="outp", bufs=3) as outp, \
             tc.tile_pool(name="psb", bufs=2, space="PSUM") as psb, \
             tc.tile_pool(name="psg", bufs=1, space="PSUM") as psg:

            iota = constp.tile([128, W], f32)
            nc.sync.dma_start(iota[:, :], iota_d[:, :])
            blkc = constp.tile([128, BAND], f16)
            nc.sync.dma_start(blkc[:, :], blkc_d[:, :])
            negk = constp.tile([128, 1], f32)
            nc.sync.dma_start(negk[:, :], negk_d[:, :])
            sels = []
            for q in range(nq):
                st = constp.tile([64, 128], f16, tag=f"sel{q}")
                nc.sync.dma_start(st[:, :], sel_d[q][:, :])
                sels.append(st)

            # x -> fp16
            xf16 = xpool.tile([128, xfree + 16], f16)
            nc.vector.memset(xf16[:, xfree:xfree + 16], 0.0)
            nchunk = 8
            csz = xfree // nchunk
            for ci in range(nchunk):
                xc = xload.tile([128, csz], f32, tag="xc")
                nc.sync.dma_start(xc[:, :], xt_d[:, ci * csz:(ci + 1) * csz])
                nc.vector.tensor_scalar(xf16[:, ci * csz:(ci + 1) * csz], xc[:, :],
                                        -1.0, None, Alu.mult)

            # RM moving operand: partitions 0-31 gi rows, 32-63 gfh rows
            RM = xpool.tile([64, RMP], f16)
            nc.vector.memset(RM[:, :], 0.0)

            tap_thunks = []
            fin_thunks = []
            tiles = [(0, min(128, nrows))]
            if nrows > 128:
                tiles.append((128, nrows - 128))

            for t0, P in tiles:
                # f32 scratch tags sA..sD reused across stages and t0-groups
                dsp = prep.tile([P, W], f32, tag="sA")
                nc.sync.dma_start(dsp[:, :], disp_d[t0:t0 + P, :])
                gx = prep.tile([P, W], f32, tag="sB")
                nc.vector.tensor_tensor(gx[:, :], iota[0:P, :], dsp[:, :], Alu.subtract)
                nc.vector.tensor_scalar(gx[:, :], gx[:, :], 2.0,
                                        float(np.float32(1.0 / 1023.0)), Alu.mult, Alu.mult)
                nc.vector.tensor_scalar(gx[:, :], gx[:, :], -1.0, None, Alu.add)
                gxp = prep.tile([P, W], f32, tag="sC")
                nc.vector.tensor_scalar(gxp[:, :], gx[:, :], 1.0, 1023.0, Alu.add, Alu.mult)
                nc.vector.tensor_scalar(gxp[:, :], gxp[:, :], 0.5, None, Alu.mult)
                pint = prep.tile([P, W], f32, tag="sD")
                nc.vector.tensor_scalar(pint[:, :], gxp[:, :], 12582912.0,
                                        -12582912.0, Alu.add, Alu.add)
                pint16 = prep.tile([P, PADW], f16, tag="pint16")
                nc.vector.tensor_copy(pint16[:, 0:W], pint[:, :])
                nc.vector.memset(pint16[:, W:PADW], PAD_VAL)
                # pfrac = gxp - pint (in place into gxp), then cast
                nc.vector.tensor_tensor(gxp[:, :], gxp[:, :], pint[:, :], Alu.subtract)
                gfh = prep.tile([P, PADW], f16, tag="gfh")
                nc.vector.tensor_copy(gfh[:, 0:W], gxp[:, :])
                nc.vector.memset(gfh[:, W:PADW], 0.0)

                # in-bounds mask (reads gx) -> f16: |gx| <= 1
                negx = prep.tile([P, W], f32, tag="sD")
                nc.vector.tensor_scalar(negx[:, :], gx[:, :], -1.0, None, Alu.mult)
                absx = prep.tile([P, W], f32, tag="negx2")
                nc.vector.tensor_tensor(absx[:, :], gx[:, :], negx[:, :], Alu.max)
                min16 = prep.tile([P, W], f16, tag="min16")
                nc.vector.tensor_scalar(min16[:, :], absx[:, :], 1.0, None, Alu.is_le)

                # giw = windowed(pint16) - blkc  (fp16)
                giw = prep.tile([P, BAND], f16, tag="giw")
                win_src = AP(pint16.tensor, pint16.offset,
                             [[PADW, P], [128, NB], [1, WIN]])
                nc.vector.tensor_tensor(
                    giw.rearrange("p (b t) -> p b t", b=NB), win_src,
                    blkc[0:P].rearrange("p (b t) -> p b t", b=NB), Alu.subtract)
                for s in range(nslot):
                    r0 = 32 * s - t0
                    if r0 < 0 or r0 >= P:
                        continue
                    nr = min(32, P - r0)
                    nc.sync.dma_start(
                        AP(RM.tensor, RM.offset + s * BAND,
                           [[RMP, nr], [1, BAND]]),
                        AP(giw.tensor, giw.offset + r0 * BAND,
                           [[BAND, nr], [1, BAND]]))
                    nc.sync.dma_start(
                        AP(RM.tensor, RM.offset + 32 * RMP + s * BAND,
                           [[RMP, nr], [WIN, NB], [1, WIN]]),
                        AP(gfh.tensor, gfh.offset + r0 * PADW,
                           [[PADW, nr], [128, NB], [1, WIN]]))

                # occlusion mask: d = floor(disp) exactly
                d0 = prep.tile([P, W], f32, tag="sB")
                nc.vector.tensor_scalar(d0[:, :], dsp[:, :], 12582912.0,
                                        -12582912.0, Alu.add, Alu.add)
                fr = prep.tile([P, W], f32, tag="sC")
                nc.vector.tensor_tensor(fr[:, :], dsp[:, :], d0[:, :], Alu.subtract)
                corr = prep.tile([P, W], f32, tag="sD")
                nc.vector.tensor_scalar(corr[:, :], fr[:, :], 0.0, None, Alu.is_lt)
                dflo = prep.tile([P, W], f32, tag="sA")
                nc.vector.tensor_tensor(dflo[:, :], d0[:, :], corr[:, :], Alu.subtract)
                tt = prep.tile([P, W], f32, tag="sB")
                nc.vector.tensor_tensor(tt[:, :], iota[0:P, :], dflo[:, :], Alu.subtract)
                t16 = prep.tile([P, W], f16, tag="t16")
                nc.vector.tensor_scalar(t16[:, :], tt[:, :], 0.0, None, Alu.max)
                mo = prep.tile([P, W], f16, tag="mo")
                if t0 == 0:
                    t16o = prep.tile([P, W], f16, tag="t16o")
                    nc.vector.tensor_copy(t16o[:, 0:W - 1], t16[:, 1:W])
                    bad = prep.tile([P, W], f16, tag="bad")
                    nc.vector.memset(bad[:, :], 0.0)
                    eqt = prep.tile([P, W], f16, tag="eqt")

                    def mk_tap(k, eqt=eqt, bad=bad, t16=t16, t16o=t16o):
                        def run():
                            L = W - k
                            if k % 2 == 0:
                                nc.vector.tensor_tensor(eqt[:, 0:L], t16[:, k:W],
                                                        t16[:, 0:L], Alu.is_equal)
                            else:
                                nc.vector.tensor_tensor(eqt[:, 0:L],
                                                        t16o[:, k - 1:W - 1],
                                                        t16[:, 0:L], Alu.is_equal)
                            nc.vector.tensor_tensor(bad[:, 0:L], bad[:, 0:L],
                                                    eqt[:, 0:L], Alu.max)
                        return run

                    for k in range(2, 64):
                        tap_thunks.append(mk_tap(k))

                    def fin_a(mo=mo, bad=bad, min16=min16, t0=t0, P=P):
                        def run():
                            nc.vector.tensor_scalar(mo[:, :], bad[:, :], 0.0, None,
                                                    Alu.is_equal)
                            msk = prep.tile([P, W], u8, tag="msk")
                            nc.vector.tensor_tensor(msk[:, :], mo[:, :], min16[:, :],
                                                    Alu.mult)
                            nc.sync.dma_start(mask_d[t0:t0 + P, :], msk[:, :])
                        return run
                    fin_thunks.append(fin_a())
                else:
                    CW = 256
                    tbp = prep.tile([128, 328], f16, tag="tbp")
                    nc.vector.memset(tbp[:, :], -5.0)
                    for c in range(4):
                        wv = min(CW + 64, W - CW * c)
                        nc.sync.dma_start(tbp[32 * c:32 * c + 32, 0:wv],
                                          t16[:, CW * c:CW * c + wv])
                    tbo = prep.tile([128, 328], f16, tag="tbo")
                    nc.vector.tensor_copy(tbo[:, 0:327], tbp[:, 1:328])
                    badb = prep.tile([128, CW], f16, tag="badb")
                    nc.vector.memset(badb[:, :], 0.0)
                    eqb = prep.tile([128, CW], f16, tag="eqb")

                    def mk_tapb(k, eqb=eqb, badb=badb, tbp=tbp, tbo=tbo):
                        def run():
                            CW = 256
                            if k % 2 == 0:
                                nc.vector.tensor_tensor(eqb[:, :], tbp[:, k:k + CW],
                                                        tbp[:, 0:CW], Alu.is_equal)
                            else:
                                nc.vector.tensor_tensor(eqb[:, :],
                                                        tbo[:, k - 1:k - 1 + CW],
                                                        tbp[:, 0:CW], Alu.is_equal)
                            nc.vector.tensor_tensor(badb[:, :], badb[:, :],
                                                    eqb[:, :], Alu.max)
                        return run

                    for k in range(2, 64):
                        tap_thunks.append(mk_tapb(k))

                    def fin_b(mo=mo, badb=badb, min16=min16, t0=t0, P=P):
                        def run():
                            mob = prep.tile([128, 256], f16, tag="mob")
                            nc.vector.tensor_scalar(mob[:, :], badb[:, :], 0.0, None,
                                                    Alu.is_equal)
                            for c in range(4):
                                nc.sync.dma_start(mo[:, 256 * c:256 * (c + 1)],
                                                  mob[32 * c:32 * c + 32, :])
                            msk = prep.tile([P, W], u8, tag="mskb")
                            nc.vector.tensor_tensor(msk[:, :], mo[:, :], min16[:, :],
                                                    Alu.mult)
                            nc.sync.dma_start(mask_d[t0:t0 + P, :], msk[:, :])
                        return run
                    fin_thunks.append(fin_b())

            # main loop (drain tap thunks between groups)
            per_g = max(1, (len(tap_thunks) + ngrp - 1) // max(1, ngrp - 2))
            for g in range(ngrp):
                gat = psg.tile([128, W], f32, tag="gat")
                for rr in range(RGRP):
                    r = g * RGRP + rr
                    q, s = r % 32, r // 32
                    bps = psb.tile([128, BAND], f32, tag="bps")
                    rm0 = s * BAND
                    for part in range(3):
                        nc.tensor.matmul(bps[:, 512 * part:512 * (part + 1)],
                                         sels[q][:, :],
                                         RM[:, rm0 + 512 * part:rm0 + 512 * (part + 1)],
                                         start=True, stop=True)
                    bsb = bandp.tile([128, BAND], f16, tag="bsb")
                    nc.scalar.activation(bsb[:, :], bps[:, :], Act.Abs,
                                         bias=negk[:, 0:1])
                    bneg = bandp.tile([128, BAND], f16, tag="bneg")
                    nc.vector.tensor_scalar(bneg[:, :], bsb[:, :], 1.0, -1.0,
                                            Alu.min, Alu.add)
                    bnegs = [bneg[:, 0:HALF], bneg[:, HALF:BAND]]
                    for b in range(NB + 1):
                        if b < NB:
                            h, b4 = b // 4, b % 4
                            xw = xf16[:, (r * NB + b) * C:(r * NB + b + 2) * C]
                            nc.tensor.matmul(
                                gat[32 * rr:32 * rr + 32, 128 * b:128 * b + 128],
                                xw, bnegs[h][:, WIN * b4:WIN * b4 + 128],
                                start=True, stop=False, skip_group_check=True,
                                tile_position=(0, 32 * rr))
                        bp = b - 1
                        if 0 <= bp < NB - 1:
                            h, b4 = bp // 4, bp % 4
                            xw = xf16[:, (r * NB + bp) * C:(r * NB + bp + 1) * C]
                            nc.tensor.matmul(
                                gat[32 * rr:32 * rr + 16,
                                    128 * bp + 128:128 * bp + 192],
                                xw, bnegs[h][:, WIN * b4 + 128:WIN * b4 + 192],
                                start=False, stop=True, skip_group_check=True,
                                tile_position=(0, 32 * rr))
                for _ in range(per_g):
                    if tap_thunks:
                        tap_thunks.pop(0)()
                osb = outp.tile([128, W], f32, tag="osb")
                if g % 2 == 0:
                    nc.scalar.activation(osb[:, :], gat[:, :], Act.Copy)
                else:
                    nc.vector.tensor_copy(osb[:, :], gat[:, :])
                for rr in range(RGRP):
                    nc.sync.dma_start(
                        AP(warp_d, (g * RGRP + rr) * W, [[nrows * W, C], [1, W]]),
                        osb[32 * rr:32 * rr + 16, :])
            while tap_thunks:
                tap_thunks.pop(0)()
            for f in fin_thunks:
                f()

    nc.compile()
    return nc


def make_consts(nrows=RPC):
    nq = min(32, nrows)
    iota = np.broadcast_to(np.arange(W, dtype=np.float32), (128, W)).copy()
    blkc = np.zeros((128, BAND), np.float16)
    for b in range(NB):
        blkc[:, b * WIN:(b + 1) * WIN] = np.float16(128 * b)
    sel = np.zeros((nq, 64, 128), np.float16)
    for q in range(nq):
        sel[q, q % 32, :] = 1.0
        sel[q, 32 + q % 32, :] = 1.0
    negk = -np.arange(128, dtype=np.float32).reshape(128, 1)
    return {"iotaj": iota, "blkc": blkc, "sel": sel, "negk": negk}


def shard_core(x, disparity, k, nrows=RPC):
    b, hh = k // 2, k % 2
    xs = x[b, :, hh * RPC:hh * RPC + nrows, :]
    dsp = disparity[b, 0, hh * RPC:hh * RPC + nrows, :]
    xt = np.ascontiguousarray(
        xs.reshape(C, nrows, NB, 128).transpose(3, 1, 2, 0)
    ).reshape(128, nrows * NB * C).astype(np.float32)
    return xt, np.ascontiguousarray(dsp).astype(np.float32)


def kernel(x, disparity):
    from concourse.bass_utils import run_bass_kernel_spmd

    x = np.asarray(x)
    disparity = np.asarray(disparity)
    if "nc" not in _cached:
        _cached["nc"] = build_program(RPC)
    nc = _cached["nc"]
    consts = make_consts(RPC)
    in_maps = []
    for k in range(NCORES):
        xt, dsp = shard_core(x, disparity, k)
        in_maps.append({"xt": xt, "disp": dsp, **consts})
    import os
    trace = bool(os.environ.get("BASS_TRACE"))
    res = run_bass_kernel_spmd(nc, in_maps, core_ids=list(range(NCORES)),
                               trace=trace)
    _cached["last_results"] = res
    warped = np.empty((N, C, H, W), np.float32)
    mask = np.empty((N, 1, H, W), bool)
    for k in range(NCORES):
        b, hh = k // 2, k % 2
        out = res.results[k]
        warped[b, :, hh * RPC:(hh + 1) * RPC, :] = out["warped"]
        mask[b, 0, hh * RPC:(hh + 1) * RPC, :] = out["mask"].astype(bool)
    return warped, mask


# revision 22
# speedup vs baseline: 1.0877x; 1.0877x over previous
"""DisparityWarping Trainium2 kernel (8-core data parallel).

Full inputs: x [4,16,320,1024] f32, disparity [4,1,320,1024] f32 in [0,64).
Returns (warped [4,16,320,1024] f32, mask [4,1,320,1024] bool).

Sharding: core k -> batch k//2, row-half k%2 (160 rows each).

Per-core algorithm:
  warp[c,i,j] = sum_m x[c,i,m] * hat(gx_pix[i,j] - m),  hat(t)=relu(1-|t|)
    - vertical bilinear is identity (gy_pix == row) to ~1.5e-5.
    - banded matmul: m-blocks of 128; j-window 192 per block.
    - band built on PE: selector matmul broadcasts gxloc = (P_int-128*blk)+frac
      over 128 partitions; ACT Abs with per-partition bias -k gives |t|;
      DVE min/sub gives -hat in fp16; gather matmuls accumulate -out in PSUM.
  mask[i,j] = (|gx|<=1) & not exists k in [2,63]: trans[i,j+k]==trans[i,j],
      trans = max(j - floor(disp), 0)   (62-tap shifted compare on DVE)
"""

import numpy as np

N, C, H, W = 4, 16, 320, 1024
RPC = 160
NB = 8
WIN = 192
BAND = NB * WIN          # 1536
HALF = BAND // 2         # 768
PADW = W + 64            # 1088
NCORES = 8
RGRP = 4
A_BIG = np.float32(8388607.5)
PAD_VAL = -10000.0

_cached = {}


def build_program(nrows=RPC):
    import concourse.bacc as bacc
    import concourse.mybir as mybir
    import concourse.tile as tile
    from concourse.bass import AP

    f32, f16, u8 = mybir.dt.float32, mybir.dt.float16, mybir.dt.uint8
    Alu = mybir.AluOpType
    Act = mybir.ActivationFunctionType

    nc = bacc.Bacc("TRN2", target_bir_lowering=False, debug=False,
                   enable_asserts=False, num_devices=NCORES)

    nslot = (nrows + 31) // 32
    nq = min(32, nrows)
    ngrp = nrows // RGRP
    xfree = nrows * NB * C
    RMP = nslot * BAND   # RM per-partition pitch

    xt_d = nc.dram_tensor("xt", [128, xfree], f32, kind="ExternalInput")
    disp_d = nc.dram_tensor("disp", [nrows, W], f32, kind="ExternalInput")
    iota_d = nc.dram_tensor("iotaj", [128, W], f32, kind="ExternalInput")
    blkc_d = nc.dram_tensor("blkc", [128, BAND], f16, kind="ExternalInput")
    sel_d = nc.dram_tensor("sel", [nq, 64, 128], f16, kind="ExternalInput")
    negk_d = nc.dram_tensor("negk", [128, 1], f32, kind="ExternalInput")
    warp_d = nc.dram_tensor("warped", [C, nrows, W], f32, kind="ExternalOutput")
    mask_d = nc.dram_tensor("mask", [nrows, W], u8, kind="ExternalOutput")

    with tile.TileContext(nc) as tc:
        with tc.tile_pool(name="const", bufs=1) as constp, \
             tc.tile_pool(name="xpool", bufs=1) as xpool, \
             tc.tile_pool(name="xload", bufs=3) as xload, \
             tc.tile_pool(name="prep", bufs=1) as prep, \
             tc.tile_pool(name="band", bufs=4) as bandp, \
             tc.tile_pool(name="outp", bufs=4) as outp, \
             tc.tile_pool(name="psb", bufs=2, space="PSUM") as psb, \
             tc.tile_pool(name="psg", bufs=1, space="PSUM") as psg:

            iota = constp.tile([128, W], f32)
            nc.sync.dma_start(iota[:, :], iota_d[:, :])
            blkc = constp.tile([128, BAND], f16)
            nc.sync.dma_start(blkc[:, :], blkc_d[:, :])
            negk = constp.tile([128, 1], f32)
            nc.sync.dma_start(negk[:, :], negk_d[:, :])
            sels = []
            for q in range(nq):
                st = constp.tile([64, 128], f16, tag=f"sel{q}")
                nc.sync.dma_start(st[:, :], sel_d[q][:, :])
                sels.append(st)

            # x -> fp16
            xf16 = xpool.tile([128, xfree + 16], f16)
            nc.vector.memset(xf16[:, xfree:xfree + 16], 0.0)
            nchunk = 8
            csz = xfree // nchunk
            cast_thunks = []

            def mk_cast(ci):
                def run():
                    xc = xload.tile([128, csz], f32, tag="xc")
                    nc.sync.dma_start(xc[:, :], xt_d[:, ci * csz:(ci + 1) * csz])
                    nc.vector.tensor_scalar(xf16[:, ci * csz:(ci + 1) * csz],
                                            xc[:, :], -1.0, None, Alu.mult)
                return run

            mk_cast(0)()
            mk_cast(1)()
            for ci in range(2, nchunk):
                cast_thunks.append(mk_cast(ci))

            # RM moving operand: partitions 0-31 gi rows, 32-63 gfh rows
            RM = xpool.tile([64, RMP], f16)
            nc.vector.memset(RM[:, :], 0.0)

            tap_thunks = list(cast_thunks)
            fin_thunks = []
            tiles = [(0, min(128, nrows))]
            if nrows > 128:
                tiles.append((128, nrows - 128))

            for t0, P in tiles:
                # f32 scratch tags sA..sD reused across stages and t0-groups
                dsp = prep.tile([P, W], f32, tag="sA")
                nc.sync.dma_start(dsp[:, :], disp_d[t0:t0 + P, :])
                gx = prep.tile([P, W], f32, tag="sB")
                nc.vector.tensor_tensor(gx[:, :], iota[0:P, :], dsp[:, :], Alu.subtract)
                nc.vector.tensor_scalar(gx[:, :], gx[:, :], 2.0,
                                        float(np.float32(1.0 / 1023.0)), Alu.mult, Alu.mult)
                nc.vector.tensor_scalar(gx[:, :], gx[:, :], -1.0, None, Alu.add)
                gxp = prep.tile([P, W], f32, tag="sC")
                nc.vector.tensor_scalar(gxp[:, :], gx[:, :], 1.0, 1023.0, Alu.add, Alu.mult)
                nc.vector.tensor_scalar(gxp[:, :], gxp[:, :], 0.5, None, Alu.mult)
                pint = prep.tile([P, W], f32, tag="sD")
                nc.vector.tensor_scalar(pint[:, :], gxp[:, :], 12582912.0,
                                        -12582912.0, Alu.add, Alu.add)
                pint16 = prep.tile([P, PADW], f16, tag="pint16")
                nc.vector.tensor_copy(pint16[:, 0:W], pint[:, :])
                nc.vector.memset(pint16[:, W:PADW], PAD_VAL)
                # pfrac = gxp - pint (in place into gxp), then cast
                nc.vector.tensor_tensor(gxp[:, :], gxp[:, :], pint[:, :], Alu.subtract)
                gfh = prep.tile([P, PADW], f16, tag="gfh")
                nc.vector.tensor_copy(gfh[:, 0:W], gxp[:, :])
                nc.vector.memset(gfh[:, W:PADW], 0.0)

                # in-bounds mask (reads gx) -> f16: |gx| <= 1
                negx = prep.tile([P, W], f32, tag="sD")
                nc.vector.tensor_scalar(negx[:, :], gx[:, :], -1.0, None, Alu.mult)
                absx = prep.tile([P, W], f32, tag="negx2")
                nc.vector.tensor_tensor(absx[:, :], gx[:, :], negx[:, :], Alu.max)
                min16 = prep.tile([P, W], f16, tag="min16")
                nc.vector.tensor_scalar(min16[:, :], absx[:, :], 1.0, None, Alu.is_le)

                # giw = windowed(pint16) - blkc  (fp16)
                giw = prep.tile([P, BAND], f16, tag="giw")
                win_src = AP(pint16.tensor, pint16.offset,
                             [[PADW, P], [128, NB], [1, WIN]])
                nc.vector.tensor_tensor(
                    giw.rearrange("p (b t) -> p b t", b=NB), win_src,
                    blkc[0:P].rearrange("p (b t) -> p b t", b=NB), Alu.subtract)
                for s in range(nslot):
                    r0 = 32 * s - t0
                    if r0 < 0 or r0 >= P:
                        continue
                    nr = min(32, P - r0)
                    nc.sync.dma_start(
                        AP(RM.tensor, RM.offset + s * BAND,
                           [[RMP, nr], [1, BAND]]),
                        AP(giw.tensor, giw.offset + r0 * BAND,
                           [[BAND, nr], [1, BAND]]))
                    nc.sync.dma_start(
                        AP(RM.tensor, RM.offset + 32 * RMP + s * BAND,
                           [[RMP, nr], [WIN, NB], [1, WIN]]),
                        AP(gfh.tensor, gfh.offset + r0 * PADW,
                           [[PADW, nr], [128, NB], [1, WIN]]))

                # occlusion mask: d = floor(disp) exactly
                d0 = prep.tile([P, W], f32, tag="sB")
                nc.vector.tensor_scalar(d0[:, :], dsp[:, :], 12582912.0,
                                        -12582912.0, Alu.add, Alu.add)
                fr = prep.tile([P, W], f32, tag="sC")
                nc.vector.tensor_tensor(fr[:, :], dsp[:, :], d0[:, :], Alu.subtract)
                corr = prep.tile([P, W], f32, tag="sD")
                nc.vector.tensor_scalar(corr[:, :], fr[:, :], 0.0, None, Alu.is_lt)
                dflo = prep.tile([P, W], f32, tag="sA")
                nc.vector.tensor_tensor(dflo[:, :], d0[:, :], corr[:, :], Alu.subtract)
                tt = prep.tile([P, W], f32, tag="sB")
                nc.vector.tensor_tensor(tt[:, :], iota[0:P, :], dflo[:, :], Alu.subtract)
                t16 = prep.tile([P, W], f16, tag="t16")
                nc.vector.tensor_scalar(t16[:, :], tt[:, :], 0.0, None, Alu.max)
                mo = prep.tile([P, W], f16, tag="mo")
                if t0 == 0:
                    t16o = prep.tile([P, W], f16, tag="t16o")
                    nc.vector.tensor_copy(t16o[:, 0:W - 1], t16[:, 1:W])
                    bad = prep.tile([P, W], f16, tag="bad")
                    nc.vector.memset(bad[:, :], 0.0)
                    eqt = prep.tile([P, W], f16, tag="eqt")

                    def mk_tap(k, eqt=eqt, bad=bad, t16=t16, t16o=t16o):
                        def run():
                            L = W - k
                            if k % 2 == 0:
                                nc.vector.tensor_tensor(eqt[:, 0:L], t16[:, k:W],
                                                        t16[:, 0:L], Alu.is_equal)
                            else:
                                nc.vector.tensor_tensor(eqt[:, 0:L],
                                                        t16o[:, k - 1:W - 1],
                                                        t16[:, 0:L], Alu.is_equal)
                            nc.vector.tensor_tensor(bad[:, 0:L], bad[:, 0:L],
                                                    eqt[:, 0:L], Alu.max)
                        return run

                    for k in range(2, 64):
                        tap_thunks.append(mk_tap(k))

                    def fin_a(mo=mo, bad=bad, min16=min16, t0=t0, P=P):
                        def run():
                            nc.vector.tensor_scalar(mo[:, :], bad[:, :], 0.0, None,
                                                    Alu.is_equal)
                            msk = prep.tile([P, W], u8, tag="msk")
                            nc.vector.tensor_tensor(msk[:, :], mo[:, :], min16[:, :],
                                                    Alu.mult)
                            nc.sync.dma_start(mask_d[t0:t0 + P, :], msk[:, :])
                        return run
                    fin_thunks.append(fin_a())
                else:
                    CW = 256
                    tbp = prep.tile([128, 328], f16, tag="tbp")
                    nc.vector.memset(tbp[:, :], -5.0)
                    for c in range(4):
                        wv = min(CW + 64, W - CW * c)
                        nc.sync.dma_start(tbp[32 * c:32 * c + 32, 0:wv],
                                          t16[:, CW * c:CW * c + wv])
                    tbo = prep.tile([128, 328], f16, tag="tbo")
                    nc.vector.tensor_copy(tbo[:, 0:327], tbp[:, 1:328])
                    badb = prep.tile([128, CW], f16, tag="badb")
                    nc.vector.memset(badb[:, :], 0.0)
                    eqb = prep.tile([128, CW], f16, tag="eqb")

                    def mk_tapb(k, eqb=eqb, badb=badb, tbp=tbp, tbo=tbo):
                        def run():
                            CW = 256
                            if k % 2 == 0:
                                nc.vector.tensor_tensor(eqb[:, :], tbp[:, k:k + CW],
                                                        tbp[:, 0:CW], Alu.is_equal)
                            else:
                                nc.vector.tensor_tensor(eqb[:, :],
                                                        tbo[:, k - 1:k - 1 + CW],
                                                        tbp[:, 0:CW], Alu.is_equal)
                            nc.vector.tensor_tensor(badb[:, :], badb[:, :],
                                                    eqb[:, :], Alu.max)
                        return run

                    for k in range(2, 64):
                        tap_thunks.append(mk_tapb(k))

                    def fin_b(mo=mo, badb=badb, min16=min16, t0=t0, P=P):
                        def run():
                            mob = prep.tile([128, 256], f16, tag="mob")
                            nc.vector.tensor_scalar(mob[:, :], badb[:, :], 0.0, None,
                                                    Alu.is_equal)
                            for c in range(4):
                                nc.sync.dma_start(mo[:, 256 * c:256 * (c + 1)],
                                                  mob[32 * c:32 * c + 32, :])
                            msk = prep.tile([P, W], u8, tag="mskb")
                            nc.vector.tensor_tensor(msk[:, :], mo[:, :], min16[:, :],
                                                    Alu.mult)
                            nc.sync.dma_start(mask_d[t0:t0 + P, :], msk[:, :])
                        return run
                    fin_thunks.append(fin_b())

            # main loop (drain tap thunks between groups)
            for g in range(ngrp):
                gat = psg.tile([128, W], f32, tag="gat")
                for rr in range(RGRP):
                    r = g * RGRP + rr
                    q, s = r % 32, r // 32
                    bps = psb.tile([128, BAND], f32, tag="bps")
                    rm0 = s * BAND
                    for part in range(3):
                        nc.tensor.matmul(bps[:, 512 * part:512 * (part + 1)],
                                         sels[q][:, :],
                                         RM[:, rm0 + 512 * part:rm0 + 512 * (part + 1)],
                                         start=True, stop=True)
                    bsb = bandp.tile([128, BAND], f16, tag="bsb")
                    nc.scalar.activation(bsb[:, :], bps[:, :], Act.Abs,
                                         bias=negk[:, 0:1])
                    bneg = bandp.tile([128, BAND], f16, tag="bneg")
                    nc.vector.tensor_scalar(bneg[:, :], bsb[:, :], 1.0, -1.0,
                                            Alu.min, Alu.add)
                    bnegs = [bneg[:, 0:HALF], bneg[:, HALF:BAND]]
                    if tap_thunks:
                        tap_thunks.pop(0)()
                    for b in range(NB + 1):
                        if b < NB:
                            h, b4 = b // 4, b % 4
                            xw = xf16[:, (r * NB + b) * C:(r * NB + b + 2) * C]
                            nc.tensor.matmul(
                                gat[32 * rr:32 * rr + 32, 128 * b:128 * b + 128],
                                xw, bnegs[h][:, WIN * b4:WIN * b4 + 128],
                                start=True, stop=False, skip_group_check=True,
                                tile_position=(0, 32 * rr))
                        bp = b - 1
                        if 0 <= bp < NB - 1:
                            h, b4 = bp // 4, bp % 4
                            xw = xf16[:, (r * NB + bp) * C:(r * NB + bp + 1) * C]
                            nc.tensor.matmul(
                                gat[32 * rr:32 * rr + 16,
                                    128 * bp + 128:128 * bp + 192],
                                xw, bnegs[h][:, WIN * b4 + 128:WIN * b4 + 192],
                                start=False, stop=True, skip_group_check=True,
                                tile_position=(0, 32 * rr))
                osb = outp.tile([128, W], f32, tag="osb")
                nc.vector.tensor_copy(osb[:, :], gat[:, :])
                for rr in range(RGRP):
                    nc.sync.dma_start(
                        AP(warp_d, (g * RGRP + rr) * W, [[nrows * W, C], [1, W]]),
                        osb[32 * rr:32 * rr + 16, :])
            while tap_thunks:
                tap_thunks.pop(0)()
            for f in fin_thunks:
                f()

    nc.compile()
    return nc


def make_consts(nrows=RPC):
    nq = min(32, nrows)
    iota = np.broadcast_to(np.arange(W, dtype=np.float32), (128, W)).copy()
    blkc = np.zeros((128, BAND), np.float16)
    for b in range(NB):
        blkc[:, b * WIN:(b + 1) * WIN] = np.float16(128 * b)
    sel = np.zeros((nq, 64, 128), np.float16)
    for q in range(nq):
        sel[q, q % 32, :] = 1.0
        sel[q, 32 + q % 32, :] = 1.0
    negk = -np.arange(128, dtype=np.float32).reshape(128, 1)
    return {"iotaj": iota, "blkc": blkc, "sel": sel, "negk": negk}


def shard_core(x, disparity, k, nrows=RPC):
    b, hh = k // 2, k % 2
    xs = x[b, :, hh * RPC:hh * RPC + nrows, :]
    dsp = disparity[b, 0, hh * RPC:hh * RPC + nrows, :]
    xt = np.ascontiguousarray(
        xs.reshape(C, nrows, NB, 128).transpose(3, 1, 2, 0)
    ).reshape(128, nrows * NB * C).astype(np.float32)
    return xt, np.ascontiguousarray(dsp).astype(np.float32)


def kernel(x, disparity):
    from concourse.bass_utils import run_bass_kernel_spmd

    x = np.asarray(x)
    disparity = np.asarray(disparity)
    if "nc" not in _cached:
        _cached["nc"] = build_program(RPC)
    nc = _cached["nc"]
    consts = make_consts(RPC)
    in_maps = []
    for k in range(NCORES):
        xt, dsp = shard_core(x, disparity, k)
        in_maps.append({"xt": xt, "disp": dsp, **consts})
    import os
    trace = bool(os.environ.get("BASS_TRACE"))
    res = run_bass_kernel_spmd(nc, in_maps, core_ids=list(range(NCORES)),
                               trace=trace)
    _cached["last_results"] = res
    warped = np.empty((N, C, H, W), np.float32)
    mask = np.empty((N, 1, H, W), bool)
    for k in range(NCORES):
        b, hh = k // 2, k % 2
        out = res.results[k]
        warped[b, :, hh * RPC:(hh + 1) * RPC, :] = out["warped"]
        mask[b, 0, hh * RPC:(hh + 1) * RPC, :] = out["mask"].astype(bool)
    return warped, mask


# revision 25
# speedup vs baseline: 1.0951x; 1.0068x over previous
"""DisparityWarping Trainium2 kernel (8-core data parallel).

Full inputs: x [4,16,320,1024] f32, disparity [4,1,320,1024] f32 in [0,64).
Returns (warped [4,16,320,1024] f32, mask [4,1,320,1024] bool).

Sharding: core k -> batch k//2, row-half k%2 (160 rows each).

Per-core algorithm:
  warp[c,i,j] = sum_m x[c,i,m] * hat(gx_pix[i,j] - m),  hat(t)=relu(1-|t|)
    - vertical bilinear is identity (gy_pix == row) to ~1.5e-5.
    - banded matmul: m-blocks of 128; j-window 192 per block.
    - band built on PE: selector matmul broadcasts gxloc = (P_int-128*blk)+frac
      over 128 partitions; ACT Abs with per-partition bias -k gives |t|;
      DVE min/sub gives -hat in fp16; gather matmuls accumulate -out in PSUM.
  mask[i,j] = (|gx|<=1) & not exists k in [2,63]: trans[i,j+k]==trans[i,j],
      trans = max(j - floor(disp), 0)   (62-tap shifted compare on DVE)
"""

import numpy as np

N, C, H, W = 4, 16, 320, 1024
RPC = 160
NB = 8
WIN = 192
BAND = NB * WIN          # 1536
HALF = BAND // 2         # 768
PADW = W + 64            # 1088
NCORES = 8
RGRP = 4
A_BIG = np.float32(8388607.5)
PAD_VAL = -10000.0

_cached = {}


def build_program(nrows=RPC):
    import concourse.bacc as bacc
    import concourse.mybir as mybir
    import concourse.tile as tile
    from concourse.bass import AP

    f32, f16, u8 = mybir.dt.float32, mybir.dt.float16, mybir.dt.uint8
    Alu = mybir.AluOpType
    Act = mybir.ActivationFunctionType

    nc = bacc.Bacc("TRN2", target_bir_lowering=False, debug=False,
                   enable_asserts=False, num_devices=NCORES)

    nslot = (nrows + 31) // 32
    nq = min(32, nrows)
    ngrp = nrows // RGRP
    xfree = nrows * NB * C
    RMP = nslot * BAND   # RM per-partition pitch

    xt_d = nc.dram_tensor("xt", [128, xfree], f32, kind="ExternalInput")
    disp_d = nc.dram_tensor("disp", [nrows, W], f32, kind="ExternalInput")
    iota_d = nc.dram_tensor("iotaj", [128, W], f32, kind="ExternalInput")
    blkc_d = nc.dram_tensor("blkc", [128, BAND], f16, kind="ExternalInput")
    sel_d = nc.dram_tensor("sel", [nq, 64, 128], f16, kind="ExternalInput")
    negk_d = nc.dram_tensor("negk", [128, 1], f32, kind="ExternalInput")
    warp_d = nc.dram_tensor("warped", [C, nrows, W], f32, kind="ExternalOutput")
    mask_d = nc.dram_tensor("mask", [nrows, W], u8, kind="ExternalOutput")

    with tile.TileContext(nc) as tc:
        with tc.tile_pool(name="const", bufs=1) as constp, \
             tc.tile_pool(name="xpool", bufs=1) as xpool, \
             tc.tile_pool(name="xload", bufs=3) as xload, \
             tc.tile_pool(name="prep", bufs=1) as prep, \
             tc.tile_pool(name="band", bufs=4) as bandp, \
             tc.tile_pool(name="outp", bufs=4) as outp, \
             tc.tile_pool(name="psb", bufs=2, space="PSUM") as psb, \
             tc.tile_pool(name="psg", bufs=1, space="PSUM") as psg:

            iota = constp.tile([128, W], f32)
            nc.sync.dma_start(iota[:, :], iota_d[:, :])
            blkc = constp.tile([128, BAND], f16)
            nc.sync.dma_start(blkc[:, :], blkc_d[:, :])
            negk = constp.tile([128, 1], f32)
            nc.sync.dma_start(negk[:, :], negk_d[:, :])
            sels = []
            for q in range(nq):
                st = constp.tile([64, 128], f16, tag=f"sel{q}")
                nc.sync.dma_start(st[:, :], sel_d[q][:, :])
                sels.append(st)

            # x -> fp16
            xf16 = xpool.tile([128, xfree + 16], f16)
            nc.vector.memset(xf16[:, xfree:xfree + 16], 0.0)
            nchunk = 8
            csz = xfree // nchunk
            cast_thunks = []

            def mk_cast(ci):
                def run():
                    xc = xload.tile([128, csz], f32, tag="xc")
                    nc.sync.dma_start(xc[:, :], xt_d[:, ci * csz:(ci + 1) * csz])
                    nc.vector.tensor_scalar(xf16[:, ci * csz:(ci + 1) * csz],
                                            xc[:, :], -1.0, None, Alu.mult)
                return run

            mk_cast(0)()
            mk_cast(1)()
            for ci in range(2, nchunk):
                cast_thunks.append(mk_cast(ci))

            # RM moving operand: partitions 0-31 gi rows, 32-63 gfh rows
            RM = xpool.tile([64, RMP], f16)
            nc.vector.memset(RM[:, :], 0.0)

            tap_thunks = list(cast_thunks)
            fin_thunks = []
            tiles = [(0, min(128, nrows))]
            if nrows > 128:
                tiles.append((128, nrows - 128))

            for t0, P in tiles:
                # f32 scratch tags sA..sD reused across stages and t0-groups
                dsp = prep.tile([P, W], f32, tag="sA")
                nc.sync.dma_start(dsp[:, :], disp_d[t0:t0 + P, :])
                gx = prep.tile([P, W], f32, tag="sB")
                nc.vector.tensor_tensor(gx[:, :], iota[0:P, :], dsp[:, :], Alu.subtract)
                nc.vector.tensor_scalar(gx[:, :], gx[:, :], 2.0,
                                        float(np.float32(1.0 / 1023.0)), Alu.mult, Alu.mult)
                nc.vector.tensor_scalar(gx[:, :], gx[:, :], -1.0, None, Alu.add)
                gxp = prep.tile([P, W], f32, tag="sC")
                nc.vector.tensor_scalar(gxp[:, :], gx[:, :], 1.0, 1023.0, Alu.add, Alu.mult)
                nc.vector.tensor_scalar(gxp[:, :], gxp[:, :], 0.5, None, Alu.mult)
                pint = prep.tile([P, W], f32, tag="sD")
                nc.vector.tensor_scalar(pint[:, :], gxp[:, :], 12582912.0,
                                        -12582912.0, Alu.add, Alu.add)
                pint16 = prep.tile([P, PADW], f16, tag="pint16")
                nc.scalar.copy(pint16[:, 0:W], pint[:, :])
                nc.vector.memset(pint16[:, W:PADW], PAD_VAL)
                # pfrac = gxp - pint (in place into gxp), then cast
                nc.vector.tensor_tensor(gxp[:, :], gxp[:, :], pint[:, :], Alu.subtract)
                gfh = prep.tile([P, PADW], f16, tag="gfh")
                nc.scalar.copy(gfh[:, 0:W], gxp[:, :])
                nc.vector.memset(gfh[:, W:PADW], 0.0)

                # in-bounds mask (reads gx) -> f16: |gx| <= 1
                negx = prep.tile([P, W], f32, tag="sD")
                nc.vector.tensor_scalar(negx[:, :], gx[:, :], -1.0, None, Alu.mult)
                absx = prep.tile([P, W], f32, tag="negx2")
                nc.vector.tensor_tensor(absx[:, :], gx[:, :], negx[:, :], Alu.max)
                min16 = prep.tile([P, W], f16, tag="min16")
                nc.vector.tensor_scalar(min16[:, :], absx[:, :], 1.0, None, Alu.is_le)

                # giw = windowed(pint16) - blkc  (fp16)
                giw = prep.tile([P, BAND], f16, tag="giw")
                win_src = AP(pint16.tensor, pint16.offset,
                             [[PADW, P], [128, NB], [1, WIN]])
                nc.vector.tensor_tensor(
                    giw.rearrange("p (b t) -> p b t", b=NB), win_src,
                    blkc[0:P].rearrange("p (b t) -> p b t", b=NB), Alu.subtract)
                for s in range(nslot):
                    r0 = 32 * s - t0
                    if r0 < 0 or r0 >= P:
                        continue
                    nr = min(32, P - r0)
                    nc.sync.dma_start(
                        AP(RM.tensor, RM.offset + s * BAND,
                           [[RMP, nr], [1, BAND]]),
                        AP(giw.tensor, giw.offset + r0 * BAND,
                           [[BAND, nr], [1, BAND]]))
                    nc.sync.dma_start(
                        AP(RM.tensor, RM.offset + 32 * RMP + s * BAND,
                           [[RMP, nr], [WIN, NB], [1, WIN]]),
                        AP(gfh.tensor, gfh.offset + r0 * PADW,
                           [[PADW, nr], [128, NB], [1, WIN]]))

                # occlusion mask: d = floor(disp) exactly
                d0 = prep.tile([P, W], f32, tag="sB")
                nc.vector.tensor_scalar(d0[:, :], dsp[:, :], 12582912.0,
                                        -12582912.0, Alu.add, Alu.add)
                fr = prep.tile([P, W], f32, tag="sC")
                nc.vector.tensor_tensor(fr[:, :], dsp[:, :], d0[:, :], Alu.subtract)
                corr = prep.tile([P, W], f32, tag="sD")
                nc.vector.tensor_scalar(corr[:, :], fr[:, :], 0.0, None, Alu.is_lt)
                dflo = prep.tile([P, W], f32, tag="sA")
                nc.vector.tensor_tensor(dflo[:, :], d0[:, :], corr[:, :], Alu.subtract)
                tt = prep.tile([P, W], f32, tag="sB")
                nc.vector.tensor_tensor(tt[:, :], iota[0:P, :], dflo[:, :], Alu.subtract)
                t16 = prep.tile([P, W], f16, tag="t16")
                nc.vector.tensor_scalar(t16[:, :], tt[:, :], 0.0, None, Alu.max)
                mo = prep.tile([P, W], f16, tag="mo")
                if t0 == 0:
                    t16o = prep.tile([P, W], f16, tag="t16o")
                    nc.scalar.copy(t16o[:, 0:W - 1], t16[:, 1:W])
                    bad = prep.tile([P, W], f16, tag="bad")
                    nc.vector.memset(bad[:, :], 0.0)
                    eqt = prep.tile([P, W], f16, tag="eqt")

                    def mk_tap(k, eqt=eqt, bad=bad, t16=t16, t16o=t16o):
                        def run():
                            L = W - k
                            if k % 2 == 0:
                                nc.vector.tensor_tensor(eqt[:, 0:L], t16[:, k:W],
                                                        t16[:, 0:L], Alu.is_equal)
                            else:
                                nc.vector.tensor_tensor(eqt[:, 0:L],
                                                        t16o[:, k - 1:W - 1],
                                                        t16[:, 0:L], Alu.is_equal)
                            nc.vector.tensor_tensor(bad[:, 0:L], bad[:, 0:L],
                                                    eqt[:, 0:L], Alu.max)
                        return run

                    for k in range(2, 64):
                        tap_thunks.append(mk_tap(k))

                    def fin_a(mo=mo, bad=bad, min16=min16, t0=t0, P=P):
                        def run():
                            nc.vector.tensor_scalar(mo[:, :], bad[:, :], 0.0, None,
                                                    Alu.is_equal)
                            msk = prep.tile([P, W], u8, tag="msk")
                            nc.vector.tensor_tensor(msk[:, :], mo[:, :], min16[:, :],
                                                    Alu.mult)
                            nc.sync.dma_start(mask_d[t0:t0 + P, :], msk[:, :])
                        return run
                    fin_thunks.append(fin_a())
                else:
                    CW = 256
                    tbp = prep.tile([128, 328], f16, tag="tbp")
                    nc.vector.memset(tbp[:, :], -5.0)
                    for c in range(4):
                        wv = min(CW + 64, W - CW * c)
                        nc.sync.dma_start(tbp[32 * c:32 * c + 32, 0:wv],
                                          t16[:, CW * c:CW * c + wv])
                    tbo = prep.tile([128, 328], f16, tag="tbo")
                    nc.vector.tensor_copy(tbo[:, 0:327], tbp[:, 1:328])
                    badb = prep.tile([128, CW], f16, tag="badb")
                    nc.vector.memset(badb[:, :], 0.0)
                    eqb = prep.tile([128, CW], f16, tag="eqb")

                    def mk_tapb(k, eqb=eqb, badb=badb, tbp=tbp, tbo=tbo):
                        def run():
                            CW = 256
                            if k % 2 == 0:
                                nc.vector.tensor_tensor(eqb[:, :], tbp[:, k:k + CW],
                                                        tbp[:, 0:CW], Alu.is_equal)
                            else:
                                nc.vector.tensor_tensor(eqb[:, :],
                                                        tbo[:, k - 1:k - 1 + CW],
                                                        tbp[:, 0:CW], Alu.is_equal)
                            nc.vector.tensor_tensor(badb[:, :], badb[:, :],
                                                    eqb[:, :], Alu.max)
                        return run

                    for k in range(2, 64):
                        tap_thunks.append(mk_tapb(k))

                    def fin_b(mo=mo, badb=badb, min16=min16, t0=t0, P=P):
                        def run():
                            mob = prep.tile([128, 256], f16, tag="mob")
                            nc.vector.tensor_scalar(mob[:, :], badb[:, :], 0.0, None,
                                                    Alu.is_equal)
                            for c in range(4):
                                nc.sync.dma_start(mo[:, 256 * c:256 * (c + 1)],
                                                  mob[32 * c:32 * c + 32, :])
                            msk = prep.tile([P, W], u8, tag="mskb")
                            nc.vector.tensor_tensor(msk[:, :], mo[:, :], min16[:, :],
                                                    Alu.mult)
                            nc.sync.dma_start(mask_d[t0:t0 + P, :], msk[:, :])
                        return run
                    fin_thunks.append(fin_b())

            # main loop (drain tap thunks between groups)
            for g in range(ngrp):
                gat = psg.tile([128, W], f32, tag="gat")
                for rr in range(RGRP):
                    r = g * RGRP + rr
                    q, s = r % 32, r // 32
                    bps = psb.tile([128, BAND], f32, tag="bps")
                    rm0 = s * BAND
                    for part in range(3):
                        nc.tensor.matmul(bps[:, 512 * part:512 * (part + 1)],
                                         sels[q][:, :],
                                         RM[:, rm0 + 512 * part:rm0 + 512 * (part + 1)],
                                         start=True, stop=True)
                    bsb = bandp.tile([128, BAND], f16, tag="bsb")
                    nc.scalar.activation(bsb[:, :], bps[:, :], Act.Abs,
                                         bias=negk[:, 0:1])
                    bneg = bandp.tile([128, BAND], f16, tag="bneg")
                    nc.vector.tensor_scalar(bneg[:, :], bsb[:, :], 1.0, -1.0,
                                            Alu.min, Alu.add)
                    bnegs = [bneg[:, 0:HALF], bneg[:, HALF:BAND]]
                    if tap_thunks:
                        tap_thunks.pop(0)()
                    for b in range(NB + 1):
                        if b < NB:
                            h, b4 = b // 4, b % 4
                            xw = xf16[:, (r * NB + b) * C:(r * NB + b + 2) * C]
                            nc.tensor.matmul(
                                gat[32 * rr:32 * rr + 32, 128 * b:128 * b + 128],
                                xw, bnegs[h][:, WIN * b4:WIN * b4 + 128],
                                start=True, stop=False, skip_group_check=True,
                                tile_position=(0, 32 * rr))
                        bp = b - 1
                        if 0 <= bp < NB - 1:
                            h, b4 = bp // 4, bp % 4
                            xw = xf16[:, (r * NB + bp) * C:(r * NB + bp + 1) * C]
                            nc.tensor.matmul(
                                gat[32 * rr:32 * rr + 16,
                                    128 * bp + 128:128 * bp + 192],
                                xw, bnegs[h][:, WIN * b4 + 128:WIN * b4 + 192],
                                start=False, stop=True, skip_group_check=True,
                                tile_position=(0, 32 * rr))
                osb = outp.tile([128, W], f32, tag="osb")
                nc.vector.tensor_copy(osb[:, :], gat[:, :])
                for rr in range(RGRP):
                    nc.sync.dma_start(
                        AP(warp_d, (g * RGRP + rr) * W, [[nrows * W, C], [1, W]]),
                        osb[32 * rr:32 * rr + 16, :])
            while tap_thunks:
                tap_thunks.pop(0)()
            for f in fin_thunks:
                f()

    nc.compile()
    return nc


def make_consts(nrows=RPC):
    nq = min(32, nrows)
    iota = np.broadcast_to(np.arange(W, dtype=np.float32), (128, W)).copy()
    blkc = np.zeros((128, BAND), np.float16)
    for b in range(NB):
        blkc[:, b * WIN:(b + 1) * WIN] = np.float16(128 * b)
    sel = np.zeros((nq, 64, 128), np.float16)
    for q in range(nq):
        sel[q, q % 32, :] = 1.0
        sel[q, 32 + q % 32, :] = 1.0
    negk = -np.arange(128, dtype=np.float32).reshape(128, 1)
    return {"iotaj": iota, "blkc": blkc, "sel": sel, "negk": negk}


def shard_core(x, disparity, k, nrows=RPC):
    b, hh = k // 2, k % 2
    xs = x[b, :, hh * RPC:hh * RPC + nrows, :]
    dsp = disparity[b, 0, hh * RPC:hh * RPC + nrows, :]
    xt = np.ascontiguousarray(
        xs.reshape(C, nrows, NB, 128).transpose(3, 1, 2, 0)
    ).reshape(128, nrows * NB * C).astype(np.float32)
    return xt, np.ascontiguousarray(dsp).astype(np.float32)


def kernel(x, disparity):
    from concourse.bass_utils import run_bass_kernel_spmd

    x = np.asarray(x)
    disparity = np.asarray(disparity)
    if "nc" not in _cached:
        _cached["nc"] = build_program(RPC)
    nc = _cached["nc"]
    consts = make_consts(RPC)
    in_maps = []
    for k in range(NCORES):
        xt, dsp = shard_core(x, disparity, k)
        in_maps.append({"xt": xt, "disp": dsp, **consts})
    import os
    trace = bool(os.environ.get("BASS_TRACE"))
    res = run_bass_kernel_spmd(nc, in_maps, core_ids=list(range(NCORES)),
                               trace=trace)
    _cached["last_results"] = res
    warped = np.empty((N, C, H, W), np.float32)
    mask = np.empty((N, 1, H, W), bool)
    for k in range(NCORES):
        b, hh = k // 2, k % 2
        out = res.results[k]
        warped[b, :, hh * RPC:(hh + 1) * RPC, :] = out["warped"]
        mask[b, 0, hh * RPC:(hh + 1) * RPC, :] = out["mask"].astype(bool)
    return warped, mask
